# revision 8
# baseline (speedup 1.0000x reference)
import sys

sys.path.insert(0, "/opt/trn_rl_repo")

import numpy as np

N_NODES = 100000
N_CORES = 8
NLOC = N_NODES // N_CORES  # 12500 nodes per core
K = 48  # padded slots per node (max degree for Poisson(16) is ~43)
COLS = 512  # matmul free dim
ST = 13  # supertiles of 1024 nodes -> 13312 >= 12500
NPAD = ST * 1024
HID = 64

LAST_EXEC_NS = None
LAST_TIME_DETAIL = None


def _silu(z):
    return z / (1.0 + np.exp(-z))


def _blockdiag(w):
    # w: [64, 64] -> [128, 128] blockdiag(w, w)
    out = np.zeros((128, 128), np.float32)
    out[:64, :64] = w
    out[64:, 64:] = w
    return out


def _io_spec(nc):
    from concourse import mybir
    import jax

    partition_name = nc.partition_id_tensor.name if nc.partition_id_tensor else None
    in_names, out_names, out_avals = [], [], []
    for alloc in nc.m.functions[0].allocations:
        if not isinstance(alloc, mybir.MemoryLocationSet):
            continue
        name = alloc.memorylocations[0].name
        if alloc.kind == "ExternalInput":
            if name != partition_name:
                in_names.append(name)
        elif alloc.kind == "ExternalOutput":
            out_names.append(name)
            out_avals.append(
                jax.core.ShapedArray(
                    tuple(alloc.tensor_shape), mybir.dt.np(alloc.dtype)
                )
            )
    return partition_name, in_names, out_names, out_avals


def _make_jit(nc, donate):
    """jit(shard_map(bass_exec)) over N_CORES devices; operands must be jit
    params in order (neuronx_cc_hook contract)."""
    import jax
    from jax.sharding import Mesh, PartitionSpec
    from jax.experimental.shard_map import shard_map
    from concourse import bass2jax

    bass2jax.install_neuronx_cc_hook()
    partition_name, in_names, out_names, out_avals = _io_spec(nc)
    in_names_full = list(in_names) + list(out_names)
    if partition_name is not None:
        in_names_full.append(partition_name)
    n_params, n_outs = len(in_names), len(out_names)

    def _body(*args):
        operands = list(args)
        if partition_name is not None:
            operands.append(bass2jax.partition_id_tensor())
        return tuple(
            bass2jax._bass_exec_p.bind(
                *operands,
                out_avals=tuple(out_avals),
                in_names=tuple(in_names_full),
                out_names=tuple(out_names),
                lowering_input_output_aliases=(),
                sim_require_finite=True,
                sim_require_nnan=True,
                nc=nc,
            )
        )

    devices = jax.devices()[:N_CORES]
    mesh = Mesh(np.asarray(devices), ("core",))
    pspec = PartitionSpec("core")
    fn = shard_map(
        _body,
        mesh=mesh,
        in_specs=(pspec,) * (n_params + n_outs),
        out_specs=(pspec,) * n_outs,
        check_rep=False,
    )
    kw = dict(keep_unused=True)
    if donate:
        kw["donate_argnums"] = tuple(range(n_params, n_params + n_outs))
    return jax.jit(fn, **kw), (mesh, pspec, in_names, out_names, out_avals)


def _build_program(reps, KT, OFFS, TOT):
    """The bass SPMD program. reps>1 repeats the whole compute body (same
    inputs/outputs) for steady-state timing via NEFF-length slope."""
    import concourse.tile as tile
    import concourse.bacc as bacc
    from concourse import mybir
    from contextlib import ExitStack

    AFT = mybir.ActivationFunctionType
    f32 = mybir.dt.float32

    nc = bacc.Bacc("TRN2", target_bir_lowering=False, debug=False,
                   num_devices=N_CORES)
    xin_d = nc.dram_tensor("xin", [2, TOT * COLS], f32, kind="ExternalInput")
    wnames = ["w1d", "f0d", "w2d", "w3d", "w4d",
              "b1s", "b2s", "b3s", "b4s", "i128"]
    wshapes = {
        "w1d": [2, 128], "f0d": [2, 128], "w2d": [128, 128],
        "w3d": [128, 128], "w4d": [128, 128],
        "b1s": [128, 1], "b2s": [128, 1], "b3s": [128, 1], "b4s": [128, 1],
        "i128": [128, 128],
    }
    wd = {n: nc.dram_tensor(n, wshapes[n], f32, kind="ExternalInput")
          for n in wnames}
    out_d = nc.dram_tensor("out", [ST, 128, COLS], f32, kind="ExternalOutput")

    with tile.TileContext(nc) as tc, ExitStack() as ctx:
        wpool = ctx.enter_context(tc.tile_pool(name="w", bufs=1))
        xpool = ctx.enter_context(tc.tile_pool(name="x", bufs=1))
        hpool = ctx.enter_context(tc.tile_pool(name="h", bufs=3))
        opool = ctx.enter_context(tc.tile_pool(name="o", bufs=2))
        ppool = ctx.enter_context(tc.tile_pool(name="ps", bufs=2, space="PSUM"))
        apool = ctx.enter_context(tc.tile_pool(name="agg", bufs=2, space="PSUM"))

        wt = {}
        for name in wd:
            t = wpool.tile(list(wd[name].shape), f32, tag=name)
            nc.sync.dma_start(t[:], wd[name].ap())
            wt[name] = t

        for _rep in range(reps):
            for t_i in range(ST):
                kt = KT[t_i]
                o = int(OFFS[t_i]) * COLS
                xt = xpool.tile([2, (K + 1) * COLS], f32, tag="xt")
                nc.sync.dma_start(
                    xt[:, : (kt + 1) * COLS],
                    xin_d.ap()[:, o : o + (kt + 1) * COLS],
                )
                agg = apool.tile([128, COLS], f32, tag="agg")
                for p in range(kt):
                    sl = xt[:, p * COLS : (p + 1) * COLS]
                    ps1 = ppool.tile([128, COLS], f32, tag="ps1")
                    nc.tensor.matmul(ps1[:], wt["w1d"][:], sl, start=True, stop=True)
                    h1 = hpool.tile([128, COLS], f32, tag="h1")
                    nc.scalar.activation(h1[:], ps1[:], AFT.Silu,
                                         bias=wt["b1s"][:], scale=1.0)
                    ps2 = ppool.tile([128, COLS], f32, tag="ps2")
                    nc.tensor.matmul(ps2[:], wt["w2d"][:], h1[:], start=True, stop=True)
                    ef = hpool.tile([128, COLS], f32, tag="ef")
                    nc.scalar.activation(ef[:], ps2[:], AFT.Silu,
                                         bias=wt["b2s"][:], scale=1.0)
                    nc.tensor.matmul(agg[:], wt["i128"][:], ef[:],
                                     start=(p == 0), stop=False)
                # pad correction: agg -= padcnt * F0
                nc.tensor.matmul(
                    agg[:], wt["f0d"][:], xt[:, kt * COLS : (kt + 1) * COLS],
                    start=False, stop=True,
                )
                aggs = hpool.tile([128, COLS], f32, tag="aggs")
                nc.scalar.copy(aggs[:], agg[:])
                ps3 = ppool.tile([128, COLS], f32, tag="ps1")
                nc.tensor.matmul(ps3[:], wt["w3d"][:], aggs[:], start=True, stop=True)
                h3 = hpool.tile([128, COLS], f32, tag="h1")
                nc.scalar.activation(h3[:], ps3[:], AFT.Silu,
                                     bias=wt["b3s"][:], scale=1.0)
                ps4 = ppool.tile([128, COLS], f32, tag="ps2")
                nc.tensor.matmul(ps4[:], wt["w4d"][:], h3[:], start=True, stop=True)
                ot = opool.tile([128, COLS], f32, tag="ot")
                nc.scalar.activation(ot[:], ps4[:], AFT.Identity,
                                     bias=wt["b4s"][:], scale=1.0)
                nc.sync.dma_start(out_d.ap()[t_i], ot[:])

    nc.compile()
    return nc


def kernel(edge_index, edge_attr, W1, b1, W2, b2, W3, b3, W4, b4):
    import os
    import time as _time
    import jax
    from jax.sharding import NamedSharding

    edge_index = np.asarray(edge_index)
    x = np.asarray(edge_attr, np.float32)[:, 0]
    W1, b1, W2, b2, W3, b3, W4, b4 = [
        np.asarray(a, np.float32) for a in (W1, b1, W2, b2, W3, b3, W4, b4)
    ]
    row = np.asarray(edge_index[0], np.int64)
    E = row.shape[0]

    # ---- host prep: per-node slot grid (pure indexing/permutation) ----
    order = np.argsort(row, kind="stable")
    rows_s = row[order]
    x_s = x[order]
    counts = np.bincount(row, minlength=N_NODES)
    assert counts.max() <= K, counts.max()
    starts = np.concatenate([[0], np.cumsum(counts)])
    rank = np.arange(E, dtype=np.int64) - starts[rows_s]
    x_grid = np.zeros((N_NODES, K), np.float32)
    x_grid[rows_s, rank] = x_s

    # pad-slot correction constant F(0) (model constant, O(64) host math)
    F0 = _silu(_silu(np.zeros((1, 1), np.float32) @ W1[None, 0] * 0 + b1) @ W2 + b2)[0]

    # weights in stacked/blockdiag form
    w1d = np.zeros((2, 128), np.float32)
    w1d[0, :64] = W1[0]
    w1d[1, 64:] = W1[0]
    f0d = np.zeros((2, 128), np.float32)
    f0d[0, :64] = -F0
    f0d[1, 64:] = -F0
    w2d, w3d, w4d = _blockdiag(W2), _blockdiag(W3), _blockdiag(W4)
    b1s = np.concatenate([b1, b1]).reshape(128, 1).astype(np.float32)
    b2s = np.concatenate([b2, b2]).reshape(128, 1).astype(np.float32)
    b3s = np.concatenate([b3, b3]).reshape(128, 1).astype(np.float32)
    b4s = np.concatenate([b4, b4]).reshape(128, 1).astype(np.float32)
    i128 = np.eye(128, dtype=np.float32)

    # per-core: sort nodes by degree; supertile t uses K_t = max degree in it
    perms, degs_sorted = [], []
    for c in range(N_CORES):
        deg_c = counts[c * NLOC : (c + 1) * NLOC]
        perm = np.argsort(deg_c, kind="stable")
        perms.append(perm)
        d = np.zeros((NPAD,), np.int64)
        d[:NLOC] = deg_c[perm]
        degs_sorted.append(d)
    KT = [max(1, int(max(degs_sorted[c][t * 1024 : (t + 1) * 1024].max()
                          for c in range(N_CORES)))) for t in range(ST)]
    OFFS = np.concatenate([[0], np.cumsum([kt + 1 for kt in KT])]).astype(int)
    TOT = int(OFFS[-1])

    xins = []
    for c in range(N_CORES):
        xg_s = np.zeros((NPAD, K), np.float32)
        xg_s[:NLOC] = x_grid[c * NLOC : (c + 1) * NLOC][perms[c]]
        deg_s = degs_sorted[c]
        xin = np.zeros((2, TOT * COLS), np.float32)
        for t_i in range(ST):
            kt = KT[t_i]
            blk = xg_s[t_i * 1024 : (t_i + 1) * 1024, :kt]  # [1024, kt]
            pcb = (kt - deg_s[t_i * 1024 : (t_i + 1) * 1024]).astype(np.float32)
            blk = blk.reshape(2, COLS, kt).transpose(0, 2, 1)  # [2, kt, COLS]
            pcb = pcb.reshape(2, COLS, 1).transpose(0, 2, 1)
            seg = np.concatenate([blk, pcb], axis=1)  # [2, kt+1, COLS]
            o = OFFS[t_i] * COLS
            xin[:, o : o + (kt + 1) * COLS] = seg.reshape(2, (kt + 1) * COLS)
        xins.append(np.ascontiguousarray(xin))

    warrs = {"w1d": w1d, "f0d": f0d, "w2d": w2d, "w3d": w3d, "w4d": w4d,
             "b1s": b1s, "b2s": b2s, "b3s": b3s, "b4s": b4s, "i128": i128}
    in_maps = [{"xin": xins[c], **warrs} for c in range(N_CORES)]

    # ---- build + run (reps=1) for the result ----
    nc1 = _build_program(1, KT, OFFS, TOT)
    run1, (mesh, pspec, in_names, out_names, out_avals) = _make_jit(nc1, donate=True)
    per_core = [[np.asarray(m[n]) for n in in_names] for m in in_maps]
    concat_in = [
        np.concatenate([per_core[c][i] for c in range(N_CORES)], axis=0)
        for i in range(len(in_names))
    ]
    concat_zeros = [
        np.zeros((N_CORES * a.shape[0], *a.shape[1:]), a.dtype) for a in out_avals
    ]
    out_arrs = run1(*concat_in, *concat_zeros)
    results = [
        {
            name: np.asarray(out_arrs[i]).reshape(N_CORES, *out_avals[i].shape)[c]
            for i, name in enumerate(out_names)
        }
        for c in range(N_CORES)
    ]

    # ---- timing: slope between reps=1 and reps=1+k programs ----
    time_iters = int(os.environ.get("BASS_TIME_ITERS", "0"))
    if time_iters > 0:
        sh = NamedSharding(mesh, pspec)
        dev_in = [jax.device_put(a, sh) for a in concat_in]
        dev_zero = [jax.device_put(z, sh) for z in concat_zeros]
        jax.block_until_ready(dev_in + dev_zero)

        def _min_wall(fn, reps=8):
            best = float("inf")
            for _ in range(reps):
                t0 = _time.perf_counter()
                o = fn(*dev_in, *dev_zero)
                jax.block_until_ready(o)
                best = min(best, _time.perf_counter() - t0)
            return best

        t1_fn, _ = _make_jit(nc1, donate=False)
        ncK = _build_program(1 + time_iters, KT, OFFS, TOT)
        tK_fn, _ = _make_jit(ncK, donate=False)
        _min_wall(t1_fn, reps=2)  # warm
        _min_wall(tK_fn, reps=2)
        t1 = _min_wall(t1_fn)
        tK = _min_wall(tK_fn)
        global LAST_EXEC_NS, LAST_TIME_DETAIL
        LAST_EXEC_NS = int((tK - t1) / time_iters * 1e9)
        LAST_TIME_DETAIL = {
            "wall_1rep_s": t1, "wall_krep_s": tK, "k": time_iters,
        }

    # ---- unstack outputs ----
    out_full = np.zeros((N_NODES, HID), np.float32)
    for c in range(N_CORES):
        r = results[c]
        oh = r["out"] if isinstance(r, dict) else r[0]
        oh = np.asarray(oh).reshape(ST, 128, COLS)
        core_nodes = np.zeros((NPAD, HID), np.float32)
        for t_i in range(ST):
            core_nodes[t_i * 1024 : t_i * 1024 + 512] = oh[t_i, :64].T
            core_nodes[t_i * 1024 + 512 : (t_i + 1) * 1024] = oh[t_i, 64:].T
        out_full[c * NLOC + perms[c]] = core_nodes[:NLOC]
    return out_full
